# revision 1
# baseline (speedup 1.0000x reference)
"""nn_LocalTransformerBlock (Swin-style shifted-window attention block).

Strategy: data-parallel over batch B=64 across the 8 NeuronCores
(jax shard_map, batch dim sharded 8 ways; small params replicated
host-side by closure). All attention is local to 7x7 windows, so each
core independently processes its 8 images. The per-core program is a
fused XLA computation (LayerNorm -> shifted-window attention with
relative-position bias + mask -> projection -> reverse shift) compiled
by the Neuron compiler.

Self-contained: hardcodes shapes B,H,W,C = 64,56,56,192, heads=6,
window 7x7, shift 3,3.
"""
import numpy as np
import jax
import jax.numpy as jnp
from jax.sharding import Mesh, PartitionSpec
from jax.experimental.shard_map import shard_map
from functools import partial

B, H, W, C = 64, 56, 56, 192
HEADS = 6
WIN = (7, 7)
SHIFT = (3, 3)
N = WIN[0] * WIN[1]  # 49
NW = (H // WIN[0]) * (W // WIN[1])  # 64 windows per image
EPS = 1e-5
NCORES = 8

_cache = {}


def _rel_pos_index():
    coords = np.stack(np.meshgrid(np.arange(WIN[0]), np.arange(WIN[1]), indexing="ij"))
    cf = coords.reshape(2, -1)
    rel = (cf[:, :, None] - cf[:, None, :]).transpose(1, 2, 0)
    rel[..., 0] += WIN[0] - 1
    rel[..., 1] += WIN[1] - 1
    rel[..., 0] *= 2 * WIN[1] - 1
    return rel.sum(-1)  # (N, N) int


def _block(x, gamma, beta, w_qkv, b_qkv, bias_hnn, w_proj, b_proj, mask_matrix):
    # x: (b_loc, H, W, C) on one core
    b = x.shape[0]
    hd = C // HEADS
    scale = hd ** -0.5

    mu = jnp.mean(x, axis=-1, keepdims=True)
    var = jnp.var(x, axis=-1, keepdims=True)
    xn = (x - mu) * jax.lax.rsqrt(var + EPS) * gamma + beta

    sx = jnp.roll(xn, shift=(-SHIFT[0], -SHIFT[1]), axis=(1, 2))

    nh, nw = H // WIN[0], W // WIN[1]
    win = sx.reshape(b, nh, WIN[0], nw, WIN[1], C).transpose(0, 1, 3, 2, 4, 5)
    win = win.reshape(-1, N, C)  # (b*NW, N, C)

    qkv = (win @ w_qkv + b_qkv).reshape(-1, N, 3, HEADS, hd).transpose(2, 0, 3, 1, 4)
    q, k, v = qkv[0], qkv[1], qkv[2]  # (b*NW, HEADS, N, hd)
    attn = jnp.einsum("bhnd,bhmd->bhnm", q * scale, k)
    attn = attn + bias_hnn[None]
    attn = attn.reshape(b, NW, HEADS, N, N) + mask_matrix[None, :, None]
    attn = jax.nn.softmax(attn.reshape(-1, HEADS, N, N), axis=-1)
    out = jnp.einsum("bhnm,bhmd->bhnd", attn, v).transpose(0, 2, 1, 3).reshape(-1, N, C)
    out = out @ w_proj + b_proj

    out = out.reshape(b, nh, nw, WIN[0], WIN[1], C).transpose(0, 1, 3, 2, 4, 5)
    out = out.reshape(b, H, W, C)
    return jnp.roll(out, shift=(SHIFT[0], SHIFT[1]), axis=(1, 2))


def _get_fn():
    if "fn" in _cache:
        return _cache["fn"]
    devices = jax.devices()[:NCORES]
    mesh = Mesh(np.asarray(devices), ("core",))
    fn = jax.jit(
        shard_map(
            _block,
            mesh=mesh,
            in_specs=(
                PartitionSpec("core"),  # x sharded over batch
                PartitionSpec(),  # gamma
                PartitionSpec(),  # beta
                PartitionSpec(),  # w_qkv
                PartitionSpec(),  # b_qkv
                PartitionSpec(),  # bias_hnn
                PartitionSpec(),  # w_proj
                PartitionSpec(),  # b_proj
                PartitionSpec(),  # mask_matrix
            ),
            out_specs=PartitionSpec("core"),
            check_rep=False,
        ),
        donate_argnums=(),
    )
    _cache["fn"] = fn
    return fn


def kernel(x, gamma, beta, w_qkv, b_qkv, rel_table, w_proj, b_proj, mask_matrix):
    x = np.asarray(x, dtype=np.float32)
    rel_table = np.asarray(rel_table, dtype=np.float32)
    # host precompute: gather the (HEADS, N, N) relative-position bias table
    rpi = _rel_pos_index()
    bias_hnn = rel_table[rpi.reshape(-1)].reshape(N, N, HEADS).transpose(2, 0, 1)
    bias_hnn = np.ascontiguousarray(bias_hnn, dtype=np.float32)

    fn = _get_fn()
    out = fn(
        jnp.asarray(x),
        jnp.asarray(np.asarray(gamma, np.float32)),
        jnp.asarray(np.asarray(beta, np.float32)),
        jnp.asarray(np.asarray(w_qkv, np.float32)),
        jnp.asarray(np.asarray(b_qkv, np.float32)),
        jnp.asarray(bias_hnn),
        jnp.asarray(np.asarray(w_proj, np.float32)),
        jnp.asarray(np.asarray(b_proj, np.float32)),
        jnp.asarray(np.asarray(mask_matrix, np.float32)),
    )
    return np.asarray(out)
